# revision 12
# baseline (speedup 1.0000x reference)
"""Trainium2 Bass kernel for the LSTM decoder problem (nn_Decoder).

Math (reference):
    h0 = latent @ W_fc.T + b_fc ;  c0 = 0 ;  x0 = obs_s[-1]
    for t in 0..13:
        gates = x @ W_ih.T + h @ W_hh.T + (b_ih + b_hh)      # [B, 4H], order i,f,g,o
        c = sig(f)*c + sig(i)*tanh(g)
        h = sig(o)*tanh(c)
        x = h @ W_mlp.T + b_mlp                              # [B, 39] -> output step t

Key algebraic folds:
  - for t>=1, x_t = W_mlp h_{t-1} + b_mlp, so
        gates_t = (W_ih W_mlp + W_hh) h_{t-1} + (b_ih + b_hh + W_ih b_mlp)
    i.e. the recurrence only needs h. W_combo := W_ih@W_mlp + W_hh  [4H, H].
  - the t=0 gate pre-activations depend only on inputs:
        A0 = W_ih x0 + W_hh (W_fc latent + b_fc) + b_ih + b_hh
    computed on the HOST and shipped as an fp16 input, so the device never
    touches x0/latent/W_ih/W_hh/W_fc.
  - b_mlp is added to the output on the host, so the device mlp is bias-free.

Device layout: batch is data-parallel over 8 cores (16384 each). Per core the
shard is 4 chunks of 1024 batch columns (2 superchunks x 2 passes); on-chip
activations live in [feature, batch-column] layout with 4 batch groups
stacked on the 128 SBUF partitions (group j on partitions 32j:32j+32), so all
elementwise/activation ops run at full 128-partition width. Gate matmuls use
block-diagonal stationary weights so one matmul serves all 4 stacked groups.

Engine split per (step, chunk): PE gate+mlp matmuls -> PSUM fp32; ACT 4 gate
activations (gate bias fused via the bias port) + tanh(c); DVE the 4 cell
elementwise ops in fp16 (2x mode, all-SBUF); Pool stages the mlp psum to fp16
SBUF for the output DMA. The cell update for chunk k is emitted after the
gate activations of chunk k+1 (1-chunk software pipeline) so the ACT engine
never stalls on the DVE cell chain.

Output is [14, NSC, 2, 78, C] fp16 = two groups' 39 pose features stacked;
the host unshuffles into [14, B, 39], adds b_mlp, and casts to fp32.
"""

import numpy as np
from contextlib import ExitStack

import concourse.bass as bass
import concourse.bacc as bacc
import concourse.tile as tile
from concourse import mybir
from concourse.bass_utils import run_bass_kernel_spmd

POSE, H, LATD = 39, 32, 16
B_TOTAL, T = 131072, 14
NCORES = 8
BS = B_TOTAL // NCORES          # 16384 batch per core
NSC = 2                         # superchunks per core
GROUPS = 4                      # batch groups stacked on partitions
C = BS // (NSC * GROUPS)        # 2048 columns per group per superchunk
NPASS = 2                       # passes (chunks) per superchunk
PW = C // NPASS                 # 1024
MMW = 512                       # matmul moving free dim (one PSUM bank)
# packed-constant column offsets (fp16 weight pack)
OW_G, OW_MLP = 0, 512
WPACK_COLS = 512 + 78

F32 = mybir.dt.float32
F16 = mybir.dt.float16
SIG = mybir.ActivationFunctionType.Sigmoid
TANH = mybir.ActivationFunctionType.Tanh
MULT = mybir.AluOpType.mult
ADD = mybir.AluOpType.add


def _build_body(ctx, tc, io, _step_schedule=tuple(range(T))):
    nc = tc.nc

    consts = ctx.enter_context(tc.tile_pool(name="consts", bufs=1))
    xin = ctx.enter_context(tc.tile_pool(name="xin", bufs=1))
    state = ctx.enter_context(tc.tile_pool(name="state", bufs=1))
    acts = ctx.enter_context(tc.tile_pool(name="acts", bufs=4))
    tmps = ctx.enter_context(tc.tile_pool(name="tmps", bufs=4))
    stg = ctx.enter_context(tc.tile_pool(name="stg", bufs=3))
    psg = ctx.enter_context(tc.tile_pool(name="psg", bufs=2, space="PSUM"))
    psm = ctx.enter_context(tc.tile_pool(name="psm", bufs=2, space="PSUM"))

    # ---- constants to SBUF ----
    wpack_sb = consts.tile([128, WPACK_COLS], F16, tag="wpack", name="wpack")
    bpack_sb = consts.tile([128, 4], F32, tag="bpack", name="bpack")
    nc.sync.dma_start(out=wpack_sb, in_=io["wpack"])
    nc.sync.dma_start(out=bpack_sb, in_=io["bpack"])
    wg_sb = [wpack_sb[:, OW_G + 128 * g : OW_G + 128 * (g + 1)] for g in range(4)]
    wmlp_sb = wpack_sb[:, OW_MLP : OW_MLP + 78]
    bgc_sb = bpack_sb

    # ---- per-superchunk persistent state (both fp16: enables DVE 2x mode) ----
    h = [state.tile([128, C], F16, tag=f"h{sc}", name=f"h{sc}") for sc in range(NSC)]
    cst = [state.tile([128, C], F16, tag=f"c{sc}", name=f"c{sc}") for sc in range(NSC)]

    # ---- t=0 gate pre-activations: host-computed, DMA straight to SBUF ----
    # io["a0"]: [4 gates, NSC, 128, C] fp16 in the stacked layout.
    a0_sb = {}
    for g in [0, 2, 3]:  # f-gate unused at t=0 (c0 = 0)
        for sc in range(NSC):
            for p in range(NPASS):
                t0 = xin.tile([128, PW], F16, tag=f"a0_{g}_{sc}_{p}", name="a0c")
                nc.sync.dma_start(
                    out=t0, in_=io["a0"][g, sc][:, p * PW : (p + 1) * PW]
                )
                a0_sb[(g, sc, p)] = t0

    units = [(sc, p) for sc in range(NSC) for p in range(NPASS)]
    stage_cur = {}

    def emit_gates(t, sc, p):
        """Gate matmuls (t>0) + 4 gate activations -> fp16 SBUF tiles."""
        cols = slice(p * PW, (p + 1) * PW)
        sig = {}
        for g in [0, 2, 3] if t == 0 else [0, 1, 2, 3]:
            a = acts.tile([128, PW], F16, tag=f"a{g}", name=f"a{g}")
            if t == 0:
                nc.scalar.activation(a, a0_sb[(g, sc, p)], TANH if g == 2 else SIG)
            else:
                ps = psg.tile([128, PW], F32, tag="psg", name="psg")
                for m in range(PW // MMW):
                    nc.tensor.matmul(
                        ps[:, m * MMW : (m + 1) * MMW],
                        lhsT=wg_sb[g],
                        rhs=h[sc][:, p * PW + m * MMW : p * PW + (m + 1) * MMW],
                        start=True,
                        stop=True,
                    )
                nc.scalar.activation(
                    a, ps, TANH if g == 2 else SIG, bias=bgc_sb[:, g : g + 1]
                )
            sig[g] = a
        return sig

    def emit_cell_mlp(t, sc, p, sig):
        """LSTM cell update (DVE fp16 2x) + tanh(c) (ACT) + mlp + staging."""
        cols = slice(p * PW, (p + 1) * PW)
        if t == 0:
            # c0 = 0 -> c1 = sig(i) * tanh(g)
            nc.vector.tensor_tensor(cst[sc][:, cols], sig[0], sig[2], MULT)
        else:
            # tanh-critical chain stays on DVE (fp16 2x); h' rides Pool
            # (the 1-chunk pipeline gives it slack before the next matmul)
            t1 = tmps.tile([128, PW], F16, tag="t1", name="t1")
            nc.vector.tensor_tensor(t1, sig[1], cst[sc][:, cols], MULT)
            t2 = tmps.tile([128, PW], F16, tag="t2", name="t2")
            nc.vector.tensor_tensor(t2, sig[0], sig[2], MULT)
            nc.vector.tensor_tensor(cst[sc][:, cols], t1, t2, ADD)
        tct = tmps.tile([128, PW], F16, tag="tc", name="tc")
        nc.scalar.activation(tct, cst[sc][:, cols], TANH)
        nc.vector.tensor_tensor(h[sc][:, cols], sig[3], tct, MULT)
        # mlp output for this pass, per group-pair (no bias: host adds)
        if p == 0:
            stage_cur[sc] = [
                stg.tile([78, C], F16, tag=f"st{sc}_{pr}", name=f"st{sc}_{pr}")
                for pr in range(2)
            ]
        for pr in range(2):
            pm = psm.tile([78, PW], F32, tag="psm", name="psm")
            for m in range(PW // MMW):
                rcols = slice(p * PW + m * MMW, p * PW + (m + 1) * MMW)
                nc.tensor.matmul(
                    pm[:, m * MMW : (m + 1) * MMW],
                    lhsT=wmlp_sb[64 * pr : 64 * (pr + 1), :],
                    rhs=h[sc][64 * pr : 64 * (pr + 1), rcols],
                    start=True,
                    stop=True,
                )
            nc.vector.tensor_scalar_add(stage_cur[sc][pr][:, cols], pm, 0.0)
        if p == NPASS - 1:
            for pr in range(2):
                nc.sync.dma_start(out=io["out"][t, sc, pr], in_=stage_cur[sc][pr])

    # ---- decode steps, cell update pipelined one chunk behind the gates ----
    pending = None
    for t in _step_schedule:
        for sc, p in units:
            sig = emit_gates(t, sc, p)
            if pending is not None:
                emit_cell_mlp(*pending)
            pending = (t, sc, p, sig)
    emit_cell_mlp(*pending)


_NC_CACHE = {}


def build_nc(mode="real"):
    """mode: "real" (grading path), "timing" (big output -> internal DRAM
    scratch + tiny external output, same HW work), "nop" (RPC-floor probe)."""
    global _NC_CACHE
    if mode in _NC_CACHE:
        return _NC_CACHE[mode]
    nc = bacc.Bacc("TRN2", target_bir_lowering=False, debug=False)
    if mode == "nop":
        tin = nc.dram_tensor("a0", [1, 4], F32, kind="ExternalInput").ap()
        tout = nc.dram_tensor("tout", [1, 4], F32, kind="ExternalOutput").ap()
        with tile.TileContext(nc) as tc:
            with ExitStack() as ctx:
                pool = ctx.enter_context(tc.tile_pool(name="p", bufs=1))
                t = pool.tile([1, 4], F32, tag="t", name="t")
                nc.sync.dma_start(out=t, in_=tin)
                nc.sync.dma_start(out=tout, in_=t)
        nc.compile()
        _NC_CACHE[mode] = nc
        return nc
    io = {
        "a0": nc.dram_tensor("a0", [4, NSC, 128, C], F16, kind="ExternalInput").ap(),
        "wpack": nc.dram_tensor("wpack", [128, WPACK_COLS], F16, kind="ExternalInput").ap(),
        "bpack": nc.dram_tensor("bpack", [128, 4], F32, kind="ExternalInput").ap(),
        "out": nc.dram_tensor(
            "out",
            [T, NSC, 2, 78, C],
            F16,
            kind="ExternalOutput" if mode == "real" else "Internal",  # noqa
        ).ap(),
    }
    reps = 1
    if isinstance(mode, tuple):
        reps = mode[1]
    if mode != "real":
        io["tout"] = nc.dram_tensor("tout", [1, 4], F32, kind="ExternalOutput").ap()
    sched = tuple(t for r in range(reps) for t in range(T))
    with tile.TileContext(nc) as tc:
        with ExitStack() as ctx:
            _build_body(ctx, tc, io, sched)
            if mode != "real":
                tpool = ctx.enter_context(tc.tile_pool(name="tp", bufs=1))
                tt = tpool.tile([1, 4], F32, tag="tt", name="tt")
                nc.vector.memset(tt, 1.0)
                nc.sync.dma_start(out=io["tout"], in_=tt)
    nc.compile()
    _NC_CACHE[mode] = nc
    return nc


def prep_inputs(obs_s, latent, W_ih, W_hh, b_ih, b_hh, W_fc, b_fc, W_mlp, b_mlp):
    """Host-side weight folding + t0 preactivation + sharding."""
    f32, f16 = np.float32, np.float16
    W_ih = np.asarray(W_ih, f32)
    W_hh = np.asarray(W_hh, f32)
    b_ih = np.asarray(b_ih, f32)
    b_hh = np.asarray(b_hh, f32)
    W_fc = np.asarray(W_fc, f32)
    b_fc = np.asarray(b_fc, f32)
    W_mlp = np.asarray(W_mlp, f32)
    b_mlp = np.asarray(b_mlp, f32)

    W_combo = W_ih @ W_mlp + W_hh                    # [4H, H]
    b_combo = b_ih + b_hh + W_ih @ b_mlp             # [4H]

    wg = np.zeros((4, 128, 128), f32)
    for g in range(4):
        for j in range(4):
            wg[g, 32 * j : 32 * (j + 1), 32 * j : 32 * (j + 1)] = W_combo[
                32 * g : 32 * (g + 1)
            ].T
    wmlp = np.zeros((128, 78), f32)
    for half in range(2):
        for j in range(2):
            wmlp[
                64 * half + 32 * j : 64 * half + 32 * (j + 1),
                39 * j : 39 * (j + 1),
            ] = W_mlp.T
    bgc = np.stack([np.tile(b_combo[32 * g : 32 * (g + 1)], 4) for g in range(4)])

    wpack = np.zeros((128, WPACK_COLS), f32)
    for g in range(4):
        wpack[:, OW_G + 128 * g : OW_G + 128 * (g + 1)] = wg[g]
    wpack[:, OW_MLP : OW_MLP + 78] = wmlp
    bpack = np.zeros((128, 4), f32)
    bpack[:, 0:4] = bgc.T

    # t0 gate pre-activations on the host: A0 = W_ih x0 + W_hh h0 + b
    x0 = np.asarray(obs_s[-1], f32)                  # [B, 39]
    h0 = np.asarray(latent, f32) @ W_fc.T + b_fc     # [B, 32]
    pre = x0 @ W_ih.T + h0 @ W_hh.T + (b_ih + b_hh)  # [B, 4H]
    preT = np.ascontiguousarray(pre.T).astype(f16)   # [128, B]

    common = {"wpack": wpack.astype(f16), "bpack": bpack}
    in_maps = []
    for c in range(NCORES):
        base = c * BS
        a0 = np.empty((4, NSC, 128, C), f16)
        for g in range(4):
            for sc in range(NSC):
                for j in range(GROUPS):
                    s = base + sc * GROUPS * C + j * C
                    a0[g, sc, 32 * j : 32 * (j + 1), :] = preT[
                        32 * g : 32 * (g + 1), s : s + C
                    ]
        m = dict(common)
        m["a0"] = a0
        in_maps.append(m)
    return in_maps


def assemble_output(per_core_out, b_mlp):
    """per_core_out: list of [T, NSC, 2, 78, C] fp16 arrays -> [T, B, 39].
    Adds b_mlp on the host (it is not applied on device)."""
    b_mlp = np.asarray(b_mlp, np.float32)
    preds = np.empty((T, B_TOTAL, POSE), np.float32)
    for c in range(NCORES):
        arr = np.asarray(per_core_out[c]).astype(np.float32)
        a = (
            arr.reshape(T, NSC, 2, 2, POSE, C)
            .transpose(0, 1, 2, 3, 5, 4)
            .reshape(T, BS, POSE)
        )
        a += b_mlp
        preds[:, c * BS : (c + 1) * BS] = a
    return preds


def kernel(obs_s, latent, W_ih, W_hh, b_ih, b_hh, W_fc, b_fc, W_mlp, b_mlp, pred_len):
    assert int(pred_len) == T, f"kernel hardcodes pred_len={T}, got {pred_len}"
    in_maps = prep_inputs(
        obs_s, latent, W_ih, W_hh, b_ih, b_hh, W_fc, b_fc, W_mlp, b_mlp
    )
    nc = build_nc()
    res = run_bass_kernel_spmd(nc, in_maps, core_ids=list(range(NCORES)))
    return assemble_output([res.results[c]["out"] for c in range(NCORES)], b_mlp)
